# revision 14
# baseline (speedup 1.0000x reference)
"""HC2MMoE Trainium2 kernel (8 NeuronCores, SPMD data-parallel).

Strategy:
  - Host: group samples by domain, round-robin each domain's samples across
    the 8 cores. Each domain d occupies a fixed column range [S_d, S_d+C_d)
    with C_d = ceil(N_d/8) on every core -> identical program on all cores
    (SPMD) with ~1% padding. x is gathered+transposed+bf16-cast on host.
  - Device (per core, BC ~ 2080 padded to a multiple of 128 = BTpad cols):
      experts (dense over all samples, feature-major, bf16):
        h1 = relu(x^T W1)  [256, BT]   h2 = relu(h1 W2) [128, BT]
        eo = h2 W3 (batch-major via lhsT=h2 chunk) -> eo_sb [128, T, 6, 10]
      gates (per-domain column ranges, feature-major):
        g = relu(x^T Gw1[d]) [64, C_d],  z = g Gw2[d] + b [6, C_d]
      z -> PE-transpose -> exp (accum denominator) -> batch-major ez
      combine (wide DVE): mmoe = (sum_e ez*eo)/den ; avg = mean_e eo
      mmoe -> PE-transpose -> towers per domain -> sigmoid -> tout
  - Outputs scattered back to original sample order on host.
"""

import sys
import math
import numpy as np

try:
    import concourse.bass as bass
except ImportError:
    sys.path.insert(0, "/opt/trn_rl_repo")
    import concourse.bass as bass

import concourse.mybir as mybir
from concourse.tile import TileContext
from concourse.vector_clock import ScopedClock
from concourse import tile as _tile_mod
from concourse.bass_utils import run_bass_kernel_spmd
import ml_dtypes

BF16 = ml_dtypes.bfloat16
F32 = mybir.dt.float32
BF = mybir.dt.bfloat16

B, IN, E, D = 16384, 1024, 6, 20
EH1, EH2, EO, GH, TH = 256, 128, 10, 64, 64
N_CORES = 8
P = 128
KT = IN // P          # 8 k-tiles of x
NCHUNK = 512          # free-dim chunk for wide matmuls


def _spill_waits(nc, maxw=1):
    """The pinned walrus rejects instructions carrying more than `maxw` sem
    waits ("Too many sync wait commands").  Split excess waits onto
    preceding same-engine NOPs."""
    ctr = 0
    for f in nc.m.functions:
        for bb in f.blocks:
            insts = list(bb.instructions)
            if not any(
                i.sync_info and i.sync_info.on_wait and len(i.sync_info.on_wait) > maxw
                for i in insts
            ):
                continue
            new = []
            for inst in insts:
                si = inst.sync_info
                waits = list(si.on_wait) if si and si.on_wait else []
                if len(waits) > maxw:
                    spill, keep = waits[:-maxw], waits[-maxw:]
                    for j in range(0, len(spill), maxw):
                        nop = mybir.InstNoOp(name=f"wspill-{ctr}", ins=[], outs=[])
                        ctr += 1
                        nop.engine = inst.engine
                        nop.sync_info = mybir.SyncInfo(
                            on_wait=spill[j : j + maxw], on_update=[]
                        )
                        new.append(nop)
                    si.on_wait = keep
                    inst.sync_info = si
                new.append(inst)
            bb.instructions[:] = new


def _chunks(total, size):
    out = []
    off = 0
    while off < total:
        out.append((off, min(size, total - off)))
        off += size
    return out


def build_program(Cs, Ss, BT, T):
    """Build the SPMD Bass program for per-domain capacities Cs, offsets Ss,
    total valid cols BT, and T 128-col tiles (BTpad = T*128)."""
    BTpad = T * P
    nc = bass.Bass()

    # ---- DRAM parameters ----
    d_xT = [
        nc.declare_dram_parameter(f"xT{k}", [P, BTpad], BF, isOutput=False)
        for k in range(KT)
    ]
    d_w1 = nc.declare_dram_parameter("w1", [P, E, KT, EH1], BF, isOutput=False)
    d_w2 = nc.declare_dram_parameter("w2", [P, E, 2, EH2], BF, isOutput=False)
    d_g1 = nc.declare_dram_parameter("g1", [P, D, KT, GH], BF, isOutput=False)
    d_sp32 = nc.declare_dram_parameter("sp32", [P, 712], F32, isOutput=False)
    d_spbf = nc.declare_dram_parameter("spbf", [P, 1480], BF, isOutput=False)

    d_tout = nc.declare_dram_parameter("tout", [BTpad], F32, isOutput=True)
    d_avg = nc.declare_dram_parameter("avg", [BTpad, EO], F32, isOutput=True)
    d_mmoe = nc.declare_dram_parameter("mmoe", [BTpad, EO], F32, isOutput=True)

    nch = _chunks(BTpad, NCHUNK)

    from contextlib import ExitStack

    with TileContext(nc) as tc, ExitStack() as ctx:
        consts = ctx.enter_context(tc.tile_pool(name="consts", bufs=1))
        acts = ctx.enter_context(tc.tile_pool(name="acts", bufs=2))
        persist = ctx.enter_context(tc.tile_pool(name="persist", bufs=1))
        psA = ctx.enter_context(tc.tile_pool(name="psA", bufs=4, space="PSUM"))
        psB = ctx.enter_context(tc.tile_pool(name="psB", bufs=3, space="PSUM"))

        # ---- load constants ----
        # Emission order = DMA priority.  First expert's weights, then x
        # k-tiles in consumption order (whole tiles: 4.3KB/partition lines
        # keep the DGE descriptor count low), then gate weights (needed
        # after expert 0), then the rest.
        # tiny constants packed into two DMAs (each dma_start costs ~600ns
        # of SP issue time; 14 separate ones delayed the critical stream)
        sp32 = consts.tile([P, 712], F32, tag="sp32")
        nc.sync.dma_start(sp32[:], d_sp32[:])
        spbf = consts.tile([P, 1480], BF, tag="spbf")
        nc.sync.dma_start(spbf[:], d_spbf[:])
        b1 = sp32[:, 0:12].rearrange("p (e m) -> p e m", e=E)
        b2 = sp32[:, 12:18]
        b3r = sp32[:, 18:498]
        id128 = sp32[:, 498:626]
        gb1 = sp32[:GH, 626:646]
        gb2 = sp32[:E, 646:666]
        tb1 = sp32[:TH, 666:686]
        tb2 = sp32[:1, 686:706]
        id6 = sp32[:E, 706:712]
        w3 = spbf[:, 0:60].rearrange("p (e o) -> p e o", e=E)
        g2 = spbf[:GH, 60:180].rearrange("p (d e) -> p d e", d=D)
        t1 = spbf[:EO, 180:1460].rearrange("p (d h) -> p d h", d=D)
        t2 = spbf[:TH, 1460:1480]

        w1 = consts.tile([P, E, KT, EH1], BF, tag="w1")
        nc.sync.dma_start(w1[:, 0], d_w1[:, 0])
        xT = [
            consts.tile([P, BTpad], BF, tag=f"xT{k}", name=f"xT{k}")
            for k in range(KT)
        ]
        for k in range(KT):
            nc.sync.dma_start(xT[k][:], d_xT[k][:])
        w2 = consts.tile([P, E, 2, EH2], BF, tag="w2")
        nc.sync.dma_start(w2[:, 0], d_w2[:, 0])
        g1 = consts.tile([P, D, KT, GH], BF, tag="g1")
        nc.sync.dma_start(g1[:], d_g1[:])
        for e in range(1, E):
            nc.sync.dma_start(w1[:, e], d_w1[:, e])
            nc.sync.dma_start(w2[:, e], d_w2[:, e])

        # ---- persistent activations ----
        eo_sb = persist.tile([P, T, E, EO], F32, tag="eo")        # expert outs, batch-major
        g_sb = persist.tile([GH, BTpad], BF, tag="gsb")           # gate hidden
        z_sb = persist.tile([E, BTpad], F32, tag="zsb")           # gate logits
        ez_sb = persist.tile([P, T, E], F32, tag="ez")            # exp(z), batch-major
        den = persist.tile([P, T], F32, tag="den")
        rden = persist.tile([P, T], F32, tag="rden")
        prod = persist.tile([P, T, E, EO], F32, tag="prod")
        mmoe_sb = persist.tile([P, T, EO], F32, tag="mmoe")
        avg_sb = persist.tile([P, T, EO], F32, tag="avg")
        mmoeT = persist.tile([EO, BTpad], BF, tag="mmoeT")        # feature-major
        th_sb = persist.tile([TH, BTpad], BF, tag="th")
        tout_sb = persist.tile([1, BTpad], F32, tag="tout")

        # pad-region hygiene: cols >= BT are never written by per-domain loops
        nc.vector.memset(z_sb[:], 0.0)
        nc.vector.memset(tout_sb[:], 0.0)

        # ---- experts (dense, feature-major; L3 batch-major) ----
        tgroups = _chunks(T, 8)
        for e in range(E):
            h1m = []
            for m in range(2):
                h1 = acts.tile([P, BTpad], BF, tag=f"h1_{m}")
                for off, size in nch:
                    ps = psA.tile([P, NCHUNK], F32, tag="big")
                    for k in range(KT):
                        nc.tensor.matmul(
                            ps[:, :size],
                            w1[:, e, k, bass.ts(m, P)],
                            xT[k][:, off : off + size],
                            start=(k == 0),
                            stop=(k == KT - 1),
                        )
                    nc.scalar.activation(
                        h1[:, off : off + size],
                        ps[:, :size],
                        mybir.ActivationFunctionType.Relu,
                        bias=b1[:, e, m : m + 1],
                    )
                h1m.append(h1)
            h2 = acts.tile([P, BTpad], BF, tag="h2")
            for off, size in nch:
                ps = psA.tile([P, NCHUNK], F32, tag="big")
                for k2 in range(2):
                    nc.tensor.matmul(
                        ps[:, :size],
                        w2[:, e, k2, :],
                        h1m[k2][:, off : off + size],
                        start=(k2 == 0),
                        stop=(k2 == 1),
                    )
                nc.scalar.activation(
                    h2[:, off : off + size],
                    ps[:, :size],
                    mybir.ActivationFunctionType.Relu,
                    bias=b2[:, e : e + 1],
                )
            # L3: per 128-col tile, batch-major out [128, 10]
            for tg0, tgn in tgroups:
                pse = psA.tile([P, 8, EO], F32, tag="big")
                for tl in range(tgn):
                    t = tg0 + tl
                    nc.tensor.matmul(
                        pse[:, tl, :],
                        h2[:, bass.ts(t, P)],
                        w3[:, e, :],
                        start=True,
                        stop=True,
                    )
                # eo = psum + Eb3[e]  (b3r holds Eb3[e] tiled 8x along free)
                nc.vector.tensor_add(
                    eo_sb[:, tg0 : tg0 + tgn, e, :],
                    pse[:, :tgn, :],
                    b3r[:, bass.ts(e, 8 * EO)].rearrange(
                        "p (t o) -> p t o", t=8
                    )[:, :tgn, :],
                )

        # ---- gates (per-domain, feature-major) ----
        for d in range(D):
            if Cs[d] == 0:
                continue
            for coff, csz in _chunks(Cs[d], NCHUNK):
                c0 = Ss[d] + coff
                ps = psB.tile([GH, NCHUNK], F32, tag="small")
                for k in range(KT):
                    nc.tensor.matmul(
                        ps[:GH, :csz],
                        g1[:, d, k, :],
                        xT[k][:, c0 : c0 + csz],
                        start=(k == 0),
                        stop=(k == KT - 1),
                    )
                nc.scalar.activation(
                    g_sb[:, c0 : c0 + csz],
                    ps[:GH, :csz],
                    mybir.ActivationFunctionType.Relu,
                    bias=gb1[:, d : d + 1],
                )
                ps2 = psB.tile([E, NCHUNK], F32, tag="small")
                nc.tensor.matmul(
                    ps2[:E, :csz],
                    g2[:, d, :],
                    g_sb[:, c0 : c0 + csz],
                    start=True,
                    stop=True,
                )
                nc.vector.tensor_scalar_add(
                    z_sb[:, c0 : c0 + csz], ps2[:E, :csz], gb2[:, d : d + 1]
                )

        # ---- z -> batch-major, exp, denominator ----
        for t in range(T):
            zt = psB.tile([P, E], F32, tag="small")
            nc.tensor.transpose(zt[:], z_sb[:, bass.ts(t, P)], id6[:])
            nc.scalar.activation(
                ez_sb[:, t, :],
                zt[:],
                mybir.ActivationFunctionType.Exp,
                accum_out=den[:, t : t + 1],
            )
        nc.vector.reciprocal(rden[:], den[:])

        # ---- combine (wide DVE ops over all tiles) ----
        nc.vector.tensor_mul(
            prod[:],
            eo_sb[:],
            ez_sb[:].broadcast_to((P, T, E, EO)),
        )
        # mmoe[p,t,o] = sum_e prod[p,t,e,o]  (iterate o-major so e is innermost)
        nc.vector.reduce_sum(
            mmoe_sb[:],
            prod[:].transpose([0, 1, 3, 2]),
            axis=mybir.AxisListType.X,
        )
        nc.vector.tensor_mul(
            mmoe_sb[:], mmoe_sb[:], rden[:].broadcast_to((P, T, EO))
        )
        nc.vector.reduce_sum(
            avg_sb[:],
            eo_sb[:].transpose([0, 1, 3, 2]),
            axis=mybir.AxisListType.X,
        )
        nc.vector.tensor_scalar_mul(avg_sb[:], avg_sb[:], 1.0 / E)

        # ---- mmoe -> feature-major ----
        for t in range(T):
            mt = psB.tile([EO, P], F32, tag="small")
            nc.tensor.transpose(mt[:], mmoe_sb[:, t, :], id128[:])
            nc.scalar.activation(
                mmoeT[:, bass.ts(t, P)], mt[:], mybir.ActivationFunctionType.Copy
            )

        # ---- towers (per-domain, feature-major) ----
        for d in range(D):
            if Cs[d] == 0:
                continue
            for coff, csz in _chunks(Cs[d], NCHUNK):
                c0 = Ss[d] + coff
                ps = psB.tile([TH, NCHUNK], F32, tag="small")
                nc.tensor.matmul(
                    ps[:TH, :csz],
                    t1[:, d, :],
                    mmoeT[:, c0 : c0 + csz],
                    start=True,
                    stop=True,
                )
                nc.scalar.activation(
                    th_sb[:, c0 : c0 + csz],
                    ps[:TH, :csz],
                    mybir.ActivationFunctionType.Relu,
                    bias=tb1[:, d : d + 1],
                )
                ps2 = psB.tile([1, NCHUNK], F32, tag="small")
                nc.tensor.matmul(
                    ps2[:1, :csz],
                    t2[:, d : d + 1],
                    th_sb[:, c0 : c0 + csz],
                    start=True,
                    stop=True,
                )
                nc.scalar.activation(
                    tout_sb[:, c0 : c0 + csz],
                    ps2[:1, :csz],
                    mybir.ActivationFunctionType.Sigmoid,
                    bias=tb2[:, d : d + 1],
                )

        # ---- outputs ----
        nc.sync.dma_start(d_tout[:].rearrange("(one n) -> one n", one=1), tout_sb[:])
        nc.sync.dma_start(
            d_avg[:].rearrange("(t p) o -> p t o", p=P), avg_sb[:]
        )
        nc.sync.dma_start(
            d_mmoe[:].rearrange("(t p) o -> p t o", p=P), mmoe_sb[:]
        )

    _spill_waits(nc)
    return nc


def _prep(x, domain_id):
    did = np.asarray(domain_id).astype(np.int64)
    idx_by_d = [np.flatnonzero(did == d) for d in range(D)]
    Cs = [(len(ix) + N_CORES - 1) // N_CORES for ix in idx_by_d]
    Ss = np.concatenate([[0], np.cumsum(Cs)]).astype(np.int64)
    BT = int(Ss[-1])
    T = (BT + P - 1) // P
    BTpad = T * P
    slots = np.zeros((N_CORES, BTpad), dtype=np.int64)
    valid = np.zeros((N_CORES, BTpad), dtype=bool)
    for d in range(D):
        ix = idx_by_d[d]
        if len(ix) == 0:
            continue
        for c in range(N_CORES):
            part = ix[c::N_CORES]
            n = len(part)
            s = int(Ss[d])
            slots[c, s : s + n] = part
            valid[c, s : s + n] = True
            if n < Cs[d]:
                slots[c, s + n : s + Cs[d]] = ix[0]
    return Cs, Ss, BT, T, BTpad, slots, valid


def kernel(
    x,
    domain_id,
    Ew1,
    Eb1,
    Ew2,
    Eb2,
    Ew3,
    Eb3,
    Gw1,
    Gb1,
    Gw2,
    Gb2,
    Tw1,
    Tb1,
    Tw2,
    Tb2,
):
    x = np.asarray(x, dtype=np.float32)
    nb = x.shape[0]
    Cs, Ss, BT, T, BTpad, slots, valid = _prep(x, domain_id)
    nc = build_program(Cs, Ss, BT, T)

    f32 = np.float32
    w1 = np.ascontiguousarray(
        np.asarray(Ew1, f32).reshape(E, KT, P, EH1).transpose(2, 0, 1, 3)
    ).astype(BF16)
    w2 = np.ascontiguousarray(
        np.asarray(Ew2, f32).reshape(E, 2, P, EH2).transpose(2, 0, 1, 3)
    ).astype(BF16)
    g1 = np.ascontiguousarray(
        np.asarray(Gw1, f32).reshape(D, KT, P, GH).transpose(2, 0, 1, 3)
    ).astype(BF16)
    sp32 = np.zeros((P, 712), dtype=f32)
    sp32[:, 0:12] = np.asarray(Eb1, f32).reshape(E, 2, P).transpose(2, 0, 1).reshape(P, 12)
    sp32[:, 12:18] = np.asarray(Eb2, f32).T
    sp32[:, 18:498] = np.broadcast_to(
        np.tile(np.asarray(Eb3, f32).reshape(E, 1, EO), (1, 8, 1)).reshape(1, 480),
        (P, 480),
    )
    sp32[:, 498:626] = np.eye(P, dtype=f32)
    sp32[:GH, 626:646] = np.asarray(Gb1, f32).T
    sp32[:E, 646:666] = np.asarray(Gb2, f32).T
    sp32[:TH, 666:686] = np.asarray(Tb1, f32).T
    sp32[:1, 686:706] = np.asarray(Tb2, f32).reshape(D, 1).T
    sp32[:E, 706:712] = np.eye(E, dtype=f32)
    spbf = np.zeros((P, 1480), dtype=BF16)
    spbf[:, 0:60] = np.asarray(Ew3, f32).transpose(1, 0, 2).reshape(P, 60).astype(BF16)
    spbf[:GH, 60:180] = np.asarray(Gw2, f32).transpose(1, 0, 2).reshape(GH, 120).astype(BF16)
    spbf[:EO, 180:1460] = np.asarray(Tw1, f32).transpose(1, 0, 2).reshape(EO, 1280).astype(BF16)
    spbf[:TH, 1460:1480] = np.asarray(Tw2, f32).reshape(D, TH).T.astype(BF16)

    shared = dict(w1=w1, w2=w2, g1=g1, sp32=sp32, spbf=spbf)
    xbf = x.astype(BF16)
    in_maps = []
    for c in range(N_CORES):
        xg = np.ascontiguousarray(xbf[slots[c]].T)  # [IN, BTpad]
        m = dict(shared)
        for k in range(KT):
            m[f"xT{k}"] = np.ascontiguousarray(xg[k * P : (k + 1) * P])
        in_maps.append(m)

    res = run_bass_kernel_spmd(nc, in_maps, core_ids=list(range(N_CORES)))

    sel_out = np.zeros((nb, 1), dtype=f32)
    sel_mmoe = np.zeros((nb, EO), dtype=f32)
    avg_out = np.zeros((nb, EO), dtype=f32)
    for c in range(N_CORES):
        v = valid[c]
        gi = slots[c][v]
        sel_out[gi, 0] = res.results[c]["tout"][v]
        sel_mmoe[gi] = res.results[c]["mmoe"][v]
        avg_out[gi] = res.results[c]["avg"][v]
    return sel_out, avg_out, sel_mmoe
